# revision 43
# baseline (speedup 1.0000x reference)
"""Trainium2 Bass kernel for nn_Attention_39049842655427.

Multi-head attention (RoPE + hard mask + soft gate mask) over 8
NeuronCores: data-parallel over batch (2) x tensor-parallel over heads
(16 heads -> 4 per core).  Each core computes q/k/v projections for its
4 heads, the head-sharded attention, and a partial output projection
(wo row-sharded); the host sums the 4 partials per batch and adds bo.

Math notes (exact up to float rounding):
  reference:  e = exp(s)*hard ; a1 = e/sum(e) ; a2 = a1*soft
              attn = a2/(sum(a2)+1e-6) ; out = attn @ v
  identity:   attn = f / (F + 1e-6*E),  f = e*hard*soft,
              F = sum(f), E = sum(e*hard)
  kernel:     g = exp(s) * M2,  M2 = hard*(soft+1e-6)
              => sum(g) = F + 1e-6*E exactly; numerator uses g instead
              of f, an O(1e-6) perturbation of attn.

Pipeline structure (single merged stream, no phase split): for each
512-wide token column s4: project K/Q/V for that column, then run the
attention for query column q4=s4 (causal => only needs k/v up to s4),
then emit the output-projection rows of column s4-1 interleaved between
attention heads.  This spreads ACT(exp)/DVE load evenly under the PE's
GEMM stream.  Scores are computed transposed (s[kv,q]) so attn@v needs
no transpose; exps are batched over two PSUM banks ([128,1024]); the
softmax denominator is one ones-matmul per head after a DVE tree
reduce; the AV accumulator is evacuated unnormalized so the PSUM bank
recycles without waiting on the reciprocal chain.  RoPE pairs are
pre-permuted so the rotation partner is a partition offset of 64
(SBUF->SBUF DMA).  Zero blocks of M2 are skipped and zero PREFIXES of
diagonal blocks are truncated to 384/256/128-wide matmuls (both exact
and data-adaptive: read from the actual mask and baked into the
compiled program; dense masks fall back to the full schedule).
"""

import math
import sys
from contextlib import ExitStack

import numpy as np
import ml_dtypes

if "/opt/trn_rl_repo" not in sys.path:
    sys.path.append("/opt/trn_rl_repo")

import concourse.bass as bass  # noqa: E402,F401
import concourse.tile as tile  # noqa: E402
from concourse import bacc, bass_isa, mybir  # noqa: E402
from concourse.bass_utils import run_bass_kernel_spmd  # noqa: E402

B, S, D, H, DK = 2, 2048, 2048, 16, 128
N_CORES = 8
HPC = 4          # heads per core
DSH = HPC * DK   # 512, d-shard per core

BF16 = ml_dtypes.bfloat16

_NC_CACHE = {}


def build_bass(s_len=S, keep=None, trunc=None):
    """Build the SPMD single-core program (same NEFF on all 8 cores)."""
    f32 = mybir.dt.float32
    bf16 = mybir.dt.bfloat16
    KC = D // 128          # contraction chunks for projections
    SQ = s_len // 512      # 512-wide q/s chunks
    NKV = s_len // 128     # 128-row kv chunks
    JQ = D // 512          # output-column chunks
    KP = KC // 4           # packed weight/x tiles per column
    NQ = NKV // 4          # kv quads
    if keep is None:
        keep = tuple(tuple(True for _ in range(NKV)) for _ in range(SQ))
    if trunc is None:
        trunc = tuple(tuple(0 for _ in range(NKV)) for _ in range(SQ))
    kept_l = {q4: [kv for kv in range(NKV) if keep[q4][kv]] for q4 in range(SQ)}
    for q4 in range(SQ):
        assert kept_l[q4], "fully masked query column not supported"
    # attn(q4) can run once all its k/v chunks are projected
    sched = {q4: max(q4, max(kept_l[q4]) // 4) for q4 in range(SQ)}
    seg_attns = {s4: [q4 for q4 in range(SQ) if sched[q4] == s4]
                 for s4 in range(SQ)}
    # causal fast path: every column's attention runs in its own segment,
    # so qT tiles can live in a small rotating per-segment pool
    inseg = all(sched[q4] == q4 for q4 in range(SQ))

    nc = bacc.Bacc("TRN2", target_bir_lowering=False, debug=False,
                   num_devices=N_CORES)

    xT = nc.dram_tensor("xT", [SQ, KP, 128, 4, 512], bf16, kind="ExternalInput").ap()
    wqT = nc.dram_tensor("wqT", [KP, 128, 4, DSH], bf16, kind="ExternalInput").ap()
    wkT = nc.dram_tensor("wkT", [KP, 128, 4, DSH], bf16, kind="ExternalInput").ap()
    wvT = nc.dram_tensor("wvT", [KP, 128, 4, DSH], bf16, kind="ExternalInput").ap()
    woT = nc.dram_tensor("woT", [DSH, D], bf16, kind="ExternalInput").ap()
    bqp = nc.dram_tensor("bqp", [128, HPC], f32, kind="ExternalInput").ap()
    bkp = nc.dram_tensor("bkp", [128, HPC], f32, kind="ExternalInput").ap()
    bvb = nc.dram_tensor("bvb", [128, DSH], f32, kind="ExternalInput").ap()
    cosp = nc.dram_tensor("cosp", [128, s_len], bf16, kind="ExternalInput").ap()
    sinp = nc.dram_tensor("sinp", [128, s_len], bf16, kind="ExternalInput").ap()
    m2t = nc.dram_tensor("m2t", [SQ, NQ, 128, 4, 512], bf16, kind="ExternalInput").ap()
    y = nc.dram_tensor("y", [s_len, D], bf16, kind="ExternalOutput").ap()

    Act = mybir.ActivationFunctionType
    inv_sqrt_dk = 1.0 / math.sqrt(DK)

    with tile.TileContext(nc) as tc:
        with (
            tc.tile_pool(name="consts", bufs=1) as consts,
            tc.tile_pool(name="kvp", bufs=1) as kvp,
            tc.tile_pool(name="wpool", bufs=1) as wpool,
            tc.tile_pool(name="xpool", bufs=2) as xpool,
            tc.tile_pool(name="m2pool", bufs=1) as m2pool,
            tc.tile_pool(name="opool", bufs=2) as opool,
            tc.tile_pool(name="rope", bufs=2) as rope,
            tc.tile_pool(name="attw", bufs=2) as attw,
            tc.tile_pool(name="accp", bufs=2) as accp,
            tc.tile_pool(name="dpool", bufs=2) as dpool,
            tc.tile_pool(name="ypool", bufs=2) as ypool,
            tc.tile_pool(name="qp", bufs=2) as qp,
            tc.tile_pool(name="ps_proj", bufs=2, space="PSUM") as ps_proj,
            tc.tile_pool(name="ps_s", bufs=2, space="PSUM") as ps_s,
            tc.tile_pool(name="ps_o", bufs=2, space="PSUM") as ps_o,
        ):
            # ---- persistent tiles ----
            warm = consts.tile([128, 512], bf16, tag="warm", name="warm")
            nc.vector.memset(warm, 0.0)
            ones_kv = consts.tile([128, 1], bf16, tag="ones_kv", name="ones_kv")
            nc.vector.memset(ones_kv, 1.0)
            kT_sb = [[kvp.tile([128, 512], bf16, tag=f"kT_{h}_{c}",
                               name=f"kT_{h}_{c}") for c in range(SQ)]
                     for h in range(HPC)]
            if inseg:
                qT_sb = None        # rotating per-segment tiles (see loop)
            else:
                qT_sb = [[kvp.tile([128, 512], bf16, tag=f"qT_{h}_{c}",
                                   name=f"qT_{h}_{c}") for c in range(SQ)]
                         for h in range(HPC)]
            cur_qT = {}             # h -> current segment's qT tile
            v_sb = [kvp.tile([128, DSH], bf16, tag=f"v_{i}", name=f"v_{i}")
                    for i in range(NKV)]
            wo_sb = [consts.tile([128, D], bf16, tag=f"wo_{h}", name=f"wo_{h}")
                     for h in range(HPC)]
            wq_sb = [wpool.tile([128, 4, DSH], bf16, tag=f"wq_{i}", name=f"wq_{i}")
                     for i in range(KP)]
            wk_sb = [wpool.tile([128, 4, DSH], bf16, tag=f"wk_{i}", name=f"wk_{i}")
                     for i in range(KP)]
            wv_sb = [wpool.tile([128, 4, DSH], bf16, tag=f"wv_{i}", name=f"wv_{i}")
                     for i in range(KP)]

            # ---- startup DMAs: 3 queues drained round-robin by the DMA
            # engines, so each queue is ordered by first-need and the
            # phases stay roughly aligned across queues ----
            xcols = {0: [xpool.tile([128, 4, 512], bf16, tag=f"x_{i}",
                                    name=f"x_{i}") for i in range(KP)]}
            cos_sb = consts.tile([128, s_len], bf16, tag="cos", name="cos")
            sin_sb = consts.tile([128, s_len], bf16, tag="sin", name="sin")
            bq_sb = consts.tile([128, HPC], f32, tag="bq", name="bq")
            bk_sb = consts.tile([128, HPC], f32, tag="bk", name="bk")
            bvb_sb = consts.tile([128, DSH], f32, tag="bvb", name="bvb")
            # phase 1: K inputs on the two HWDGE queues, consts on gpsimd
            for i in range(KP):
                nc.sync.dma_start(xcols[0][i][:], xT[0, i])
                nc.scalar.dma_start(wk_sb[i][:], wkT[i])
            nc.gpsimd.dma_start(bq_sb[:], bqp[:])
            nc.gpsimd.dma_start(bk_sb[:], bkp[:])
            nc.gpsimd.dma_start(cos_sb[:], cosp[:])
            nc.gpsimd.dma_start(sin_sb[:], sinp[:])
            # phase 2: Q weights split across the HWDGE queues; the rest on
            # gpsimd ordered by first need (keeps the sync queue short so the
            # latency-critical RoPE swap DMAs aren't stuck behind bulk loads)
            nc.sync.dma_start(wq_sb[0][:], wqT[0])
            nc.sync.dma_start(wq_sb[2][:], wqT[2])
            nc.scalar.dma_start(wq_sb[1][:], wqT[1])
            nc.scalar.dma_start(wq_sb[3][:], wqT[3])
            nc.gpsimd.dma_start(bvb_sb[:], bvb[:])
            for i in range(KP):
                nc.gpsimd.dma_start(wv_sb[i][:], wvT[i])

            m2_tiles = {}  # q4 -> {quad: tile}

            def load_m2(q4, eng=None):
                eng = eng or nc.gpsimd
                m2_tiles[q4] = {}
                for i in range(NQ):
                    if any(keep[q4][4 * i + j] for j in range(4)):
                        t = m2pool.tile([128, 4, 512], bf16, tag=f"m2_{i}",
                                        name=f"m2_{i}")
                        eng.dma_start(t[:], m2t[q4, i])
                        m2_tiles[q4][i] = t

            # phase 4: first attention masks + output weights
            for q4 in seg_attns[0]:
                load_m2(q4, eng=nc.gpsimd)
            for h in range(HPC):
                nc.gpsimd.dma_start(wo_sb[h][:], woT[h * 128:(h + 1) * 128, :])

            # ---- HAM warm-up: open the PE clock gate while DMAs land ----
            ps_warm = ps_proj.tile([128, 512], f32, tag="ps_proj", name="ps_proj")
            for i in range(12):
                nc.tensor.matmul(ps_warm[:], warm[:, 0:128], warm[:],
                                 start=(i == 0), stop=(i == 11))

            oT_sb = {}

            def emit_attention_head(q4, h, y_queue=()):
                m2c = m2_tiles[q4]
                qT = cur_qT[h] if inseg else qT_sb[h][q4]
                # pieces: (kv, zero-prefix offset). Build groups:
                #   fp: tile-adjacent pairs of full-width pieces
                #   fs: leftover full singles (first one seeds the accS chain)
                #   tg: truncated pieces packed into <=1024-wide psum groups
                pieces = [(kv, trunc[q4][kv]) for kv in kept_l[q4]]
                fulls = [kv for kv, t in pieces if t == 0]
                truncs = [(kv, t) for kv, t in pieces if t > 0]
                fp, fs = [], []
                i = 0
                while i < len(fulls):
                    if (i + 1 < len(fulls) and fulls[i + 1] == fulls[i] + 1
                            and fulls[i] // 4 == fulls[i + 1] // 4):
                        fp.append([(fulls[i], 0), (fulls[i + 1], 0)])
                        i += 2
                    else:
                        fs.append([(fulls[i], 0)])
                        i += 1
                # pack truncated pieces into the two 512-wide PSUM banks of a
                # score tile: a matmul output must not cross a bank boundary
                tg = []   # each: ([items], [remA, remB])
                for kv, t in sorted(truncs, key=lambda p: p[1]):
                    w = 512 - t
                    placed = False
                    if tg:
                        items, rem = tg[-1]
                        for bnk in (0, 1):
                            if rem[bnk] >= w:
                                items.append((kv, t, 512 * bnk + 512 - rem[bnk], w))
                                rem[bnk] -= w
                                placed = True
                                break
                    if not placed:
                        tg.append(([(kv, t, 0, w)], [512 - w, 512]))
                assert fulls, "query column with no full-width kv block"
                fp = [[(a, 0, 0, 512), (b, 0, 512, 512)] for (a, _), (b, _) in fp]
                fs = [[(g[0][0], 0, 0, 512)] for g in fs]
                tg = [items for items, _ in tg]
                seq = fp[:1] + fs + tg + fp[1:]
                n_av = len(pieces)
                ng = len(seq)
                # positions (after which score-group) to slot a y chunk so the
                # PE always has ready work while exp/mul latency drains
                ypos = {max(1, math.ceil(ng * k / 4)) for k in (1, 2, 3, 4)}
                ps_oT = ps_o.tile([128, 512], f32, tag="ps_o", name="ps_o")
                accA = accS = None          # pair chain / single+trunc chain
                av_i = 0
                pend = []      # accumulator adds deferred to lag behind AV
                pend_av = []   # AV emission lagged one group behind scores
                pair_i = 0

                def emit_av():
                    nonlocal av_i
                    g2, lay = pend_av.pop(0)
                    for kv, off, c, w in lay:
                        nc.tensor.matmul(
                            ps_oT[:, off:512],
                            v_sb[kv][:, h * 128:(h + 1) * 128],
                            g2[:, c:c + w],
                            start=(av_i == 0), stop=(av_i == n_av - 1))
                        av_i += 1

                for gi, grp in enumerate(seq):
                    lay = grp  # (kv, off, psum col, width) per piece
                    tot = max(c + w for _, _, c, w in lay)
                    is_fpair = (len(grp) == 2 and grp[0][1] == 0
                                and grp[1][1] == 0)
                    psc = ps_s.tile([128, 1024], f32, tag="ps_s", name="ps_s")
                    for kv, off, c, w in lay:
                        nc.tensor.matmul(
                            psc[:, c:c + w],
                            kT_sb[h][kv // 4][:, (kv % 4) * 128:(kv % 4 + 1) * 128],
                            qT[:, off:512], start=True, stop=True)
                    if is_fpair:
                        if accA is None:
                            g = accA = accp.tile([128, 1024], bf16, tag="gaccA",
                                                 name="gaccA")
                            chain = None
                        else:
                            g = attw.tile([128, 1024], bf16,
                                          tag=f"g{pair_i % 2}", name="g")
                            chain = 'A'
                        pair_i += 1
                    elif len(grp) == 1 and grp[0][1] == 0:
                        if accS is None:
                            g = accS = accp.tile([128, 512], bf16, tag="gaccS",
                                                 name="gaccS")
                            chain = None
                        else:
                            g = attw.tile([128, 512], bf16, tag="gs", name="gs")
                            chain = 'S'
                    else:
                        g = attw.tile([128, 1024], bf16,
                                      tag=f"g{pair_i % 2}", name="g")
                        pair_i += 1
                        chain = 'T'
                    nc.scalar.activation(g[:, 0:tot], psc[:, 0:tot], Act.Exp,
                                         scale=inv_sqrt_dk)
                    if is_fpair and grp[1][0] == grp[0][0] + 1:
                        kv0 = grp[0][0]
                        nc.vector.tensor_mul(
                            g[:, 0:1024], g[:, 0:1024],
                            m2c[kv0 // 4][:, kv0 % 4:kv0 % 4 + 2, :])
                    else:
                        for kv, off, c, w in lay:
                            nc.vector.tensor_mul(
                                g[:, c:c + w], g[:, c:c + w],
                                m2c[kv // 4][:, kv % 4, off:512])
                    pend_av.append((g, lay))
                    if gi + 1 in ypos and y_queue:
                        emit_y_chunk(*y_queue.pop(0))
                    if len(pend_av) > 1:
                        emit_av()
                    if chain is not None:
                        pend.append((g, lay, chain))
                    # drain pending accumulator adds (lag keeps AV unblocked)
                    while len(pend) > 1 or (gi == ng - 1 and pend):
                        g2, lay2, ch = pend.pop(0)
                        if ch == 'A':
                            nc.vector.tensor_add(accA[:], accA[:], g2[:])
                        elif ch == 'S':
                            nc.vector.tensor_add(accS[:], accS[:], g2[:])
                        else:
                            for kv, off, c, w in lay2:
                                nc.vector.tensor_add(
                                    accS[:, off:512], accS[:, off:512],
                                    g2[:, c:c + w])
                while pend_av:
                    emit_av()
                # evacuate the AV accumulator immediately (frees the PSUM
                # bank for the next head without waiting on the denominator
                # chain); normalization happens later from SBUF
                oT_u = attw.tile([128, 512], bf16, tag="otu", name="otu")
                if h % 2 == 0:
                    nc.scalar.copy(oT_u[:], ps_oT[:])
                else:
                    nc.vector.tensor_copy(oT_u[:], ps_oT[:])
                # fold chains into one [128,512] bf16 row-block sum, then a
                # single ones-matmul computes the denominator row (1 PE
                # instruction per head; psum slot borrowed from ps_s pool)
                gfold = dpool.tile([128, 512], bf16, tag="gfold", name="gfold")
                if accA is not None:
                    nc.vector.tensor_add(gfold[:], accA[:, 0:512],
                                         accA[:, 512:1024])
                    if accS is not None:
                        nc.vector.tensor_add(gfold[:], gfold[:], accS[:])
                else:
                    nc.vector.tensor_copy(gfold[:], accS[:])
                ps_dt = ps_s.tile([128, 1024], f32, tag="ps_s", name="ps_s")
                nc.tensor.matmul(ps_dt[0:1, 0:512], ones_kv[:], gfold[:],
                                 start=True, stop=True)
                r_row = dpool.tile([1, 512], f32, tag="r_row", name="r_row")
                nc.vector.reciprocal_approx_fast(r_row[:], ps_dt[0:1, 0:512])
                rb = dpool.tile([128, 512], f32, tag="rb", name="rb")
                nc.gpsimd.partition_broadcast(rb[:], r_row[:])
                oT = opool.tile([128, 512], bf16, tag=f"oT_{h}", name=f"oT_{h}")
                nc.vector.tensor_mul(oT[:], oT_u[:], rb[:])
                oT_sb.setdefault(q4, {})[h] = oT

            yc_n = [0]

            def emit_y_chunk(q4, sl, j4):
                srow = slice((q4 * 4 + sl) * 128, (q4 * 4 + sl + 1) * 128)
                jcol = slice(j4 * 512, (j4 + 1) * 512)
                ps_y = ps_proj.tile([128, 512], f32, tag="ps_proj",
                                    name="ps_proj")
                for h in range(HPC):
                    nc.tensor.matmul(
                        ps_y[:], oT_sb[q4][h][:, sl * 128:(sl + 1) * 128],
                        wo_sb[h][:, jcol], start=(h == 0), stop=(h == HPC - 1))
                yc_n[0] += 1
                ych = ypool.tile([128, 512], bf16, tag=f"ych{yc_n[0] % 2}",
                                 name="ych")
                if yc_n[0] % 2 == 0:
                    nc.scalar.copy(ych[:], ps_y[:])
                else:
                    nc.vector.tensor_copy(ych[:], ps_y[:])
                nc.sync.dma_start(y[srow, jcol], ych[:])

            # ================= merged pipeline over columns =================
            y_queue = []
            for s4 in range(SQ):
                scol = slice(s4 * 512, (s4 + 1) * 512)
                xcol = xcols.pop(s4)

                def proj_mms(ps, w_sb, mm, ks):
                    for k in ks:
                        nc.tensor.matmul(
                            ps[:],
                            w_sb[k // 4][:, k % 4, mm * 128:(mm + 1) * 128],
                            xcol[k // 4][:, k % 4, :],
                            start=(k == 0), stop=(k == KC - 1))

                def rope_tail(ps, b_sb, mm, dtile):
                    q1 = rope.tile([128, 512], bf16, tag="q1", name="q1")
                    nc.scalar.activation(q1[:], ps[:], Act.Identity,
                                         bias=b_sb[:, mm:mm + 1])
                    # pair-swap halves via SBUF->SBUF DMA (partition
                    # shifts are not expressible on DVE/ACT lanes)
                    qsw = rope.tile([128, 512], bf16, tag="qsw", name="qsw")
                    nc.sync.dma_start(qsw[0:64], q1[64:128])
                    nc.sync.dma_start(qsw[64:128], q1[0:64])
                    tsw = rope.tile([128, 512], bf16, tag="tsw", name="tsw")
                    nc.vector.tensor_mul(tsw[:], qsw[:], sin_sb[:, scol])
                    # qsw is dead after tsw; reuse it for the cosine term
                    nc.vector.tensor_mul(qsw[:], q1[:], cos_sb[:, scol])
                    nc.vector.tensor_add(dtile[:], qsw[:], tsw[:])

                def q_dtile(mm, is_q):
                    if not is_q:
                        return kT_sb[mm][s4]
                    if inseg:
                        cur_qT[mm] = qp.tile([128, 512], bf16, tag=f"qTc_{mm}",
                                             name=f"qTc_{mm}")
                        return cur_qT[mm]
                    return qT_sb[mm][s4]

                # K then Q: out[dk, s] with RoPE (K first: scores read kT)
                for (w_sb, b_sb, is_q) in ((wk_sb, bk_sb, False),
                                           (wq_sb, bq_sb, True)):
                    if s4 == 0 and not is_q:
                        # startup: half-contraction interleave lets the PE
                        # begin with 2MB in SBUF instead of 4MB
                        ps_h = {}
                        for mm, half in ((0, 0), (1, 0), (0, 1), (2, 0),
                                         (1, 1), (3, 0), (2, 1), (3, 1)):
                            if half == 0:
                                ps_h[mm] = ps_proj.tile([128, 512], f32,
                                                        tag="ps_proj",
                                                        name="ps_proj")
                                proj_mms(ps_h[mm], w_sb, mm, range(KC // 2))
                            else:
                                proj_mms(ps_h[mm], w_sb, mm, range(KC // 2, KC))
                                rope_tail(ps_h.pop(mm), b_sb, mm,
                                          q_dtile(mm, is_q))
                        continue
                    for mm in range(HPC):
                        ps = ps_proj.tile([128, 512], f32, tag="ps_proj",
                                          name="ps_proj")
                        proj_mms(ps, w_sb, mm, range(KC))
                        rope_tail(ps, b_sb, mm, q_dtile(mm, is_q))

                # next column's x prefetch: issued after the K/Q swap DMAs so
                # the latency-critical RoPE swaps aren't queued behind 2MB
                if s4 + 1 < SQ:
                    xcols[s4 + 1] = [xpool.tile([128, 4, 512], bf16,
                                                tag=f"x_{i}", name=f"x_{i}")
                                     for i in range(KP)]
                    for i in range(KP):
                        nc.sync.dma_start(xcols[s4 + 1][i][:], xT[s4 + 1, i])

                # V: out[s, dk-shard], natural layout
                for sl in range(4):
                    s16 = s4 * 4 + sl
                    ps = ps_proj.tile([128, 512], f32, tag="ps_proj",
                                      name="ps_proj")
                    for k in range(KC):
                        nc.tensor.matmul(
                            ps[:],
                            xcol[k // 4][:, k % 4, sl * 128:(sl + 1) * 128],
                            wv_sb[k // 4][:, k % 4, :],
                            start=(k == 0), stop=(k == KC - 1))
                    nc.vector.tensor_add(v_sb[s16][:], ps[:], bvb_sb[:])

                # attention for columns whose k/v just became complete,
                # with previous columns' output rows interleaved between
                # heads to keep the PE fed during exp/mul latency.
                for q4 in seg_attns[s4]:
                    for h in range(HPC):
                        emit_attention_head(q4, h, y_queue)
                    for sl in range(4):
                        for j4 in range(JQ):
                            y_queue.append((q4, sl, j4))

                # prefetch m2 for the next segment's attention columns
                # (emitted last so the in-order queues never stall on it)
                if s4 + 1 < SQ:
                    for q4 in seg_attns[s4 + 1]:
                        load_m2(q4)

            while y_queue:
                emit_y_chunk(*y_queue.pop(0))

    nc.compile()
    return nc


def _rope_perm():
    """Within each head's 128 rows: evens first, then odds."""
    base = np.concatenate([np.arange(0, 128, 2), np.arange(1, 128, 2)])
    return np.concatenate([h * 128 + base for h in range(HPC)])


def _blk(a):
    """[R, C] -> [C//512, R//512, 128, 4, 512] packed contiguous blocks.

    Block [c4, i, :, j, :] = a[(4*i+j)*128:(4*i+j+1)*128, c4*512:(c4+1)*512].
    """
    r, c = a.shape
    return np.ascontiguousarray(
        a.reshape(r // 512, 4, 128, c // 512, 512).transpose(3, 0, 2, 1, 4))


def _wpack(a):
    """[R, C] -> [R//512, 128, 4, C]: pack 4 row-chunks per tile."""
    r, c = a.shape
    return np.ascontiguousarray(
        a.reshape(r // 512, 4, 128, c).transpose(0, 2, 1, 3))


def prepare_inputs(x, freqs, hard_mask, soft_mask, wq, bq, wk, bk, wv, bv, wo,
                   s_len=S):
    """Host-side shard + layout prep.  Returns one in_map per core."""
    perm = _rope_perm()
    cos = np.cos(np.asarray(freqs, np.float32))   # [S, 64]
    sin = np.sin(np.asarray(freqs, np.float32))
    cosp = np.ascontiguousarray(
        np.concatenate([cos.T, cos.T], axis=0)).astype(BF16)     # [128, S]
    sinp = np.ascontiguousarray(
        np.concatenate([-sin.T, sin.T], axis=0)).astype(BF16)
    hard = np.asarray(hard_mask, np.float32).reshape(s_len, s_len)
    soft = np.asarray(soft_mask, np.float32).reshape(s_len, s_len)
    m2t = _blk((hard * (soft + 1e-6)).T.astype(BF16))

    xT = [_blk(np.asarray(x[b], np.float32).T.astype(BF16)) for b in range(B)]

    per_group = []
    for hg in range(4):
        rows = slice(DSH * hg, DSH * (hg + 1))
        wq_sh = np.asarray(wq, np.float32)[rows][perm]
        wk_sh = np.asarray(wk, np.float32)[rows][perm]
        wv_sh = np.asarray(wv, np.float32)[rows]
        per_group.append({
            "wqT": _wpack(np.ascontiguousarray(wq_sh.T).astype(BF16)),
            "wkT": _wpack(np.ascontiguousarray(wk_sh.T).astype(BF16)),
            "wvT": _wpack(np.ascontiguousarray(wv_sh.T).astype(BF16)),
            "woT": np.ascontiguousarray(
                np.asarray(wo, np.float32)[:, rows].T).astype(BF16),
            "bqp": np.ascontiguousarray(
                np.asarray(bq, np.float32)[rows][perm].reshape(HPC, 128).T),
            "bkp": np.ascontiguousarray(
                np.asarray(bk, np.float32)[rows][perm].reshape(HPC, 128).T),
            "bvb": np.ascontiguousarray(np.broadcast_to(
                np.asarray(bv, np.float32)[rows][None, :], (128, DSH))),
        })

    in_maps = []
    for core in range(N_CORES):
        b, hg = core // 4, core % 4
        m = {"xT": xT[b], "cosp": cosp, "sinp": sinp, "m2t": m2t}
        m.update(per_group[hg])
        in_maps.append(m)
    return in_maps


def kernel(x, freqs, hard_mask, soft_mask, wq, bq, wk, bk, wv, bv, wo, bo,
           _trace=False, _tmpdir=None):
    s_len = x.shape[1]
    in_maps = prepare_inputs(x, freqs, hard_mask, soft_mask, wq, bq, wk, bk,
                             wv, bv, wo, s_len=s_len)
    m2b = in_maps[0]["m2t"]  # [SQ, NKV//4, 128, 4, 512]
    keep = []
    trunc = []
    for q4 in range(m2b.shape[0]):
        krow, trow = [], []
        for kv in range(m2b.shape[1] * 4):
            blk = m2b[q4, kv // 4, :, kv % 4]
            nz = np.flatnonzero(np.any(blk != 0, axis=0))
            krow.append(nz.size > 0)
            # exact zero-prefix width, 128-col granularity
            trow.append(int(nz[0]) // 128 * 128 if nz.size else 0)
        if not any(k and t == 0 for k, t in zip(krow, trow)):
            trow = [0] * len(trow)   # need one full-width block per column
        keep.append(tuple(krow))
        trunc.append(tuple(trow))
    keep, trunc = tuple(keep), tuple(trunc)
    ckey = (s_len, keep, trunc)
    if ckey not in _NC_CACHE:
        _NC_CACHE[ckey] = build_bass(s_len, keep, trunc)
    nc = _NC_CACHE[ckey]
    kwargs = {}
    if _trace:
        kwargs = {"trace": True, "tmpdir": _tmpdir}
    res = run_bass_kernel_spmd(nc, in_maps, core_ids=list(range(N_CORES)),
                               **kwargs)
    bo32 = np.asarray(bo, np.float32)
    out = np.empty((B, s_len, D), np.float32)
    for b in range(B):
        acc = res.results[4 * b]["y"].astype(np.float32)
        for hg in range(1, 4):
            acc = acc + res.results[4 * b + hg]["y"].astype(np.float32)
        out[b] = acc + bo32[None, :]
    kernel.last_result = res
    return out


# revision 44
# speedup vs baseline: 1.1630x; 1.1630x over previous
"""Trainium2 Bass kernel for nn_Attention_39049842655427.

Multi-head attention (RoPE + hard mask + soft gate mask) over 8
NeuronCores: data-parallel over batch (2) x tensor-parallel over heads
(16 heads -> 4 per core).  Each core computes q/k/v projections for its
4 heads, the head-sharded attention, and a partial output projection
(wo row-sharded); the host sums the 4 partials per batch and adds bo.

Math notes (exact up to float rounding):
  reference:  e = exp(s)*hard ; a1 = e/sum(e) ; a2 = a1*soft
              attn = a2/(sum(a2)+1e-6) ; out = attn @ v
  identity:   attn = f / (F + 1e-6*E),  f = e*hard*soft,
              F = sum(f), E = sum(e*hard)
  kernel:     g = exp(s) * M2,  M2 = hard*(soft+1e-6)
              => sum(g) = F + 1e-6*E exactly; numerator uses g instead
              of f, an O(1e-6) perturbation of attn.

Pipeline structure (single merged stream, no phase split): for each
512-wide token column s4: project K/Q/V for that column, then run the
attention for query column q4=s4 (causal => only needs k/v up to s4),
then emit the output-projection rows of column s4-1 interleaved between
attention heads.  This spreads ACT(exp)/DVE load evenly under the PE's
GEMM stream.  Scores are computed transposed (s[kv,q]) so attn@v needs
no transpose; exps are batched over two PSUM banks ([128,1024]); the
softmax denominator is one ones-matmul per head after a DVE tree
reduce; the AV accumulator is evacuated unnormalized so the PSUM bank
recycles without waiting on the reciprocal chain.  RoPE pairs are
pre-permuted so the rotation partner is a partition offset of 64
(SBUF->SBUF DMA).  Zero blocks of M2 are skipped and zero PREFIXES of
diagonal blocks are truncated to 384/256/128-wide matmuls (both exact
and data-adaptive: read from the actual mask and baked into the
compiled program; dense masks fall back to the full schedule).
"""

import math
import sys
from contextlib import ExitStack

import numpy as np
import ml_dtypes

if "/opt/trn_rl_repo" not in sys.path:
    sys.path.append("/opt/trn_rl_repo")

import concourse.bass as bass  # noqa: E402,F401
import concourse.tile as tile  # noqa: E402
from concourse import bacc, bass_isa, mybir  # noqa: E402
from concourse.bass_utils import run_bass_kernel_spmd  # noqa: E402

B, S, D, H, DK = 2, 2048, 2048, 16, 128
N_CORES = 8
HPC = 4          # heads per core
DSH = HPC * DK   # 512, d-shard per core

BF16 = ml_dtypes.bfloat16

_NC_CACHE = {}


def build_bass(s_len=S, keep=None, trunc=None):
    """Build the SPMD single-core program (same NEFF on all 8 cores)."""
    f32 = mybir.dt.float32
    bf16 = mybir.dt.bfloat16
    KC = D // 128          # contraction chunks for projections
    SQ = s_len // 512      # 512-wide q/s chunks
    NKV = s_len // 128     # 128-row kv chunks
    JQ = D // 512          # output-column chunks
    KP = KC // 4           # packed weight/x tiles per column
    NQ = NKV // 4          # kv quads
    if keep is None:
        keep = tuple(tuple(True for _ in range(NKV)) for _ in range(SQ))
    if trunc is None:
        trunc = tuple(tuple(0 for _ in range(NKV)) for _ in range(SQ))
    kept_l = {q4: [kv for kv in range(NKV) if keep[q4][kv]] for q4 in range(SQ)}
    for q4 in range(SQ):
        assert kept_l[q4], "fully masked query column not supported"
    # attn(q4) can run once all its k/v chunks are projected
    sched = {q4: max(q4, max(kept_l[q4]) // 4) for q4 in range(SQ)}
    seg_attns = {s4: [q4 for q4 in range(SQ) if sched[q4] == s4]
                 for s4 in range(SQ)}
    # causal fast path: every column's attention runs in its own segment,
    # so qT tiles can live in a small rotating per-segment pool
    inseg = all(sched[q4] == q4 for q4 in range(SQ))

    nc = bacc.Bacc("TRN2", target_bir_lowering=False, debug=False,
                   num_devices=N_CORES)

    xT = nc.dram_tensor("xT", [SQ, KP, 128, 4, 512], bf16, kind="ExternalInput").ap()
    wqT = nc.dram_tensor("wqT", [KP, 128, 4, DSH], bf16, kind="ExternalInput").ap()
    wkT = nc.dram_tensor("wkT", [KP, 128, 4, DSH], bf16, kind="ExternalInput").ap()
    wvT = nc.dram_tensor("wvT", [KP, 128, 4, DSH], bf16, kind="ExternalInput").ap()
    woT = nc.dram_tensor("woT", [DSH, D], bf16, kind="ExternalInput").ap()
    bqp = nc.dram_tensor("bqp", [128, HPC], f32, kind="ExternalInput").ap()
    bkp = nc.dram_tensor("bkp", [128, HPC], f32, kind="ExternalInput").ap()
    bvb = nc.dram_tensor("bvb", [128, DSH], f32, kind="ExternalInput").ap()
    cosp = nc.dram_tensor("cosp", [128, s_len], bf16, kind="ExternalInput").ap()
    sinp = nc.dram_tensor("sinp", [128, s_len], bf16, kind="ExternalInput").ap()
    m2t = nc.dram_tensor("m2t", [SQ, NQ, 128, 4, 512], bf16, kind="ExternalInput").ap()
    y = nc.dram_tensor("y", [s_len, D], bf16, kind="ExternalOutput").ap()

    Act = mybir.ActivationFunctionType
    inv_sqrt_dk = 1.0 / math.sqrt(DK)

    with tile.TileContext(nc) as tc:
        with (
            tc.tile_pool(name="consts", bufs=1) as consts,
            tc.tile_pool(name="kvp", bufs=1) as kvp,
            tc.tile_pool(name="wpool", bufs=1) as wpool,
            tc.tile_pool(name="xpool", bufs=2) as xpool,
            tc.tile_pool(name="m2pool", bufs=1) as m2pool,
            tc.tile_pool(name="opool", bufs=2) as opool,
            tc.tile_pool(name="rope", bufs=2) as rope,
            tc.tile_pool(name="attw", bufs=2) as attw,
            tc.tile_pool(name="accp", bufs=2) as accp,
            tc.tile_pool(name="dpool", bufs=2) as dpool,
            tc.tile_pool(name="ypool", bufs=2) as ypool,
            tc.tile_pool(name="qp", bufs=2) as qp,
            tc.tile_pool(name="ps_proj", bufs=2, space="PSUM") as ps_proj,
            tc.tile_pool(name="ps_s", bufs=2, space="PSUM") as ps_s,
            tc.tile_pool(name="ps_o", bufs=2, space="PSUM") as ps_o,
        ):
            # ---- persistent tiles ----
            warm = consts.tile([128, 512], bf16, tag="warm", name="warm")
            nc.vector.memset(warm, 0.0)
            ones_kv = consts.tile([128, 1], bf16, tag="ones_kv", name="ones_kv")
            nc.vector.memset(ones_kv, 1.0)
            kT_sb = [[kvp.tile([128, 512], bf16, tag=f"kT_{h}_{c}",
                               name=f"kT_{h}_{c}") for c in range(SQ)]
                     for h in range(HPC)]
            if inseg:
                qT_sb = None        # rotating per-segment tiles (see loop)
            else:
                qT_sb = [[kvp.tile([128, 512], bf16, tag=f"qT_{h}_{c}",
                                   name=f"qT_{h}_{c}") for c in range(SQ)]
                         for h in range(HPC)]
            cur_qT = {}             # h -> current segment's qT tile
            v_sb = [kvp.tile([128, DSH], bf16, tag=f"v_{i}", name=f"v_{i}")
                    for i in range(NKV)]
            wo_sb = [consts.tile([128, D], bf16, tag=f"wo_{h}", name=f"wo_{h}")
                     for h in range(HPC)]
            wq_sb = [wpool.tile([128, 4, DSH], bf16, tag=f"wq_{i}", name=f"wq_{i}")
                     for i in range(KP)]
            wk_sb = [wpool.tile([128, 4, DSH], bf16, tag=f"wk_{i}", name=f"wk_{i}")
                     for i in range(KP)]
            wv_sb = [wpool.tile([128, 4, DSH], bf16, tag=f"wv_{i}", name=f"wv_{i}")
                     for i in range(KP)]

            # ---- startup DMAs: 3 queues drained round-robin by the DMA
            # engines, so each queue is ordered by first-need and the
            # phases stay roughly aligned across queues ----
            xcols = {0: [xpool.tile([128, 4, 512], bf16, tag=f"x_{i}",
                                    name=f"x_{i}") for i in range(KP)]}
            cos_sb = consts.tile([128, s_len], bf16, tag="cos", name="cos")
            sin_sb = consts.tile([128, s_len], bf16, tag="sin", name="sin")
            bq_sb = consts.tile([128, HPC], f32, tag="bq", name="bq")
            bk_sb = consts.tile([128, HPC], f32, tag="bk", name="bk")
            bvb_sb = consts.tile([128, DSH], f32, tag="bvb", name="bvb")
            # phase 1: K inputs on the two HWDGE queues, consts on gpsimd
            for i in range(KP):
                nc.sync.dma_start(xcols[0][i][:], xT[0, i])
                nc.scalar.dma_start(wk_sb[i][:], wkT[i])
            nc.gpsimd.dma_start(bq_sb[:], bqp[:])
            nc.gpsimd.dma_start(bk_sb[:], bkp[:])
            nc.gpsimd.dma_start(cos_sb[:], cosp[:])
            nc.gpsimd.dma_start(sin_sb[:], sinp[:])
            # phase 2: Q weights split across the HWDGE queues; the rest on
            # gpsimd ordered by first need (keeps the sync queue short so the
            # latency-critical RoPE swap DMAs aren't stuck behind bulk loads)
            nc.sync.dma_start(wq_sb[0][:], wqT[0])
            nc.sync.dma_start(wq_sb[2][:], wqT[2])
            nc.scalar.dma_start(wq_sb[1][:], wqT[1])
            nc.scalar.dma_start(wq_sb[3][:], wqT[3])
            nc.gpsimd.dma_start(bvb_sb[:], bvb[:])
            for i in range(KP):
                nc.gpsimd.dma_start(wv_sb[i][:], wvT[i])

            m2_tiles = {}  # q4 -> {quad: tile}

            def load_m2(q4, eng=None):
                eng = eng or nc.gpsimd
                m2_tiles[q4] = {}
                for i in range(NQ):
                    if any(keep[q4][4 * i + j] for j in range(4)):
                        t = m2pool.tile([128, 4, 512], bf16, tag=f"m2_{i}",
                                        name=f"m2_{i}")
                        eng.dma_start(t[:], m2t[q4, i])
                        m2_tiles[q4][i] = t

            # phase 4: first attention masks + output weights
            for q4 in seg_attns[0]:
                load_m2(q4, eng=nc.gpsimd)
            for h in range(HPC):
                nc.gpsimd.dma_start(wo_sb[h][:], woT[h * 128:(h + 1) * 128, :])

            # ---- HAM warm-up: open the PE clock gate while DMAs land ----
            ps_warm = ps_proj.tile([128, 512], f32, tag="ps_proj", name="ps_proj")
            for i in range(12):
                nc.tensor.matmul(ps_warm[:], warm[:, 0:128], warm[:],
                                 start=(i == 0), stop=(i == 11))

            oT_sb = {}

            def emit_attention_head(q4, h, y_queue=()):
                m2c = m2_tiles[q4]
                qT = cur_qT[h] if inseg else qT_sb[h][q4]
                # pieces: (kv, zero-prefix offset). Build groups:
                #   fp: tile-adjacent pairs of full-width pieces
                #   fs: leftover full singles (first one seeds the accS chain)
                #   tg: truncated pieces packed into <=1024-wide psum groups
                pieces = [(kv, trunc[q4][kv]) for kv in kept_l[q4]]
                fulls = [kv for kv, t in pieces if t == 0]
                truncs = [(kv, t) for kv, t in pieces if t > 0]
                fp, fs = [], []
                i = 0
                while i < len(fulls):
                    if (i + 1 < len(fulls) and fulls[i + 1] == fulls[i] + 1
                            and fulls[i] // 4 == fulls[i + 1] // 4):
                        fp.append([(fulls[i], 0), (fulls[i + 1], 0)])
                        i += 2
                    else:
                        fs.append([(fulls[i], 0)])
                        i += 1
                # pack truncated pieces into the two 512-wide PSUM banks of a
                # score tile: a matmul output must not cross a bank boundary
                tg = []   # each: ([items], [remA, remB])
                for kv, t in sorted(truncs, key=lambda p: p[1]):
                    w = 512 - t
                    placed = False
                    if tg:
                        items, rem = tg[-1]
                        for bnk in (0, 1):
                            if rem[bnk] >= w:
                                items.append((kv, t, 512 * bnk + 512 - rem[bnk], w))
                                rem[bnk] -= w
                                placed = True
                                break
                    if not placed:
                        tg.append(([(kv, t, 0, w)], [512 - w, 512]))
                assert fulls, "query column with no full-width kv block"
                fp = [[(a, 0, 0, 512), (b, 0, 512, 512)] for (a, _), (b, _) in fp]
                fs = [[(g[0][0], 0, 0, 512)] for g in fs]
                tg = [items for items, _ in tg]
                seq = fp[:1] + fs + tg + fp[1:]
                n_av = len(pieces)
                ng = len(seq)
                # positions (after which score-group) to slot a y chunk so the
                # PE always has ready work while exp/mul latency drains
                ypos = {max(1, math.ceil(ng * k / 4)) for k in (1, 2, 3, 4)}
                ps_oT = ps_o.tile([128, 512], f32, tag="ps_o", name="ps_o")
                accA = accS = None          # pair chain / single+trunc chain
                av_i = 0
                pend = []      # accumulator adds deferred to lag behind AV
                pend_av = []   # AV emission lagged one group behind scores
                pair_i = 0

                def emit_av():
                    nonlocal av_i
                    g2, lay = pend_av.pop(0)
                    for kv, off, c, w in lay:
                        nc.tensor.matmul(
                            ps_oT[:, off:512],
                            v_sb[kv][:, h * 128:(h + 1) * 128],
                            g2[:, c:c + w],
                            start=(av_i == 0), stop=(av_i == n_av - 1))
                        av_i += 1

                for gi, grp in enumerate(seq):
                    lay = grp  # (kv, off, psum col, width) per piece
                    tot = max(c + w for _, _, c, w in lay)
                    is_fpair = (len(grp) == 2 and grp[0][1] == 0
                                and grp[1][1] == 0)
                    psc = ps_s.tile([128, 1024], f32, tag="ps_s", name="ps_s")
                    for kv, off, c, w in lay:
                        nc.tensor.matmul(
                            psc[:, c:c + w],
                            kT_sb[h][kv // 4][:, (kv % 4) * 128:(kv % 4 + 1) * 128],
                            qT[:, off:512], start=True, stop=True)
                    if is_fpair:
                        if accA is None:
                            g = accA = accp.tile([128, 1024], bf16, tag="gaccA",
                                                 name="gaccA")
                            chain = None
                        else:
                            g = attw.tile([128, 1024], bf16,
                                          tag=f"g{pair_i % 2}", name="g")
                            chain = 'A'
                        pair_i += 1
                    elif len(grp) == 1 and grp[0][1] == 0:
                        if accS is None:
                            g = accS = accp.tile([128, 512], bf16, tag="gaccS",
                                                 name="gaccS")
                            chain = None
                        else:
                            g = attw.tile([128, 512], bf16, tag="gs", name="gs")
                            chain = 'S'
                    else:
                        g = attw.tile([128, 1024], bf16,
                                      tag=f"g{pair_i % 2}", name="g")
                        pair_i += 1
                        chain = 'T'
                    nc.scalar.activation(g[:, 0:tot], psc[:, 0:tot], Act.Exp,
                                         scale=inv_sqrt_dk)
                    if is_fpair and grp[1][0] == grp[0][0] + 1:
                        kv0 = grp[0][0]
                        nc.vector.tensor_mul(
                            g[:, 0:1024], g[:, 0:1024],
                            m2c[kv0 // 4][:, kv0 % 4:kv0 % 4 + 2, :])
                    else:
                        for kv, off, c, w in lay:
                            nc.vector.tensor_mul(
                                g[:, c:c + w], g[:, c:c + w],
                                m2c[kv // 4][:, kv % 4, off:512])
                    pend_av.append((g, lay))
                    if gi + 1 in ypos and y_queue:
                        emit_y_chunk(*y_queue.pop(0))
                    if len(pend_av) > 1:
                        emit_av()
                    if chain is not None:
                        pend.append((g, lay, chain))
                    # drain pending accumulator adds (lag keeps AV unblocked)
                    while len(pend) > 1 or (gi == ng - 1 and pend):
                        g2, lay2, ch = pend.pop(0)
                        if ch == 'A':
                            nc.vector.tensor_add(accA[:], accA[:], g2[:])
                        elif ch == 'S':
                            nc.vector.tensor_add(accS[:], accS[:], g2[:])
                        else:
                            for kv, off, c, w in lay2:
                                nc.vector.tensor_add(
                                    accS[:, off:512], accS[:, off:512],
                                    g2[:, c:c + w])
                while pend_av:
                    emit_av()
                # evacuate the AV accumulator immediately (frees the PSUM
                # bank for the next head without waiting on the denominator
                # chain); normalization happens later from SBUF
                oT_u = attw.tile([128, 512], bf16, tag="otu", name="otu")
                if h % 2 == 0:
                    nc.scalar.copy(oT_u[:], ps_oT[:])
                else:
                    nc.vector.tensor_copy(oT_u[:], ps_oT[:])
                # fold chains into one [128,512] bf16 row-block sum, then a
                # single ones-matmul computes the denominator row (1 PE
                # instruction per head; psum slot borrowed from ps_s pool)
                gfold = dpool.tile([128, 512], bf16, tag="gfold", name="gfold")
                if accA is not None:
                    nc.vector.tensor_add(gfold[:], accA[:, 0:512],
                                         accA[:, 512:1024])
                    if accS is not None:
                        nc.vector.tensor_add(gfold[:], gfold[:], accS[:])
                else:
                    nc.vector.tensor_copy(gfold[:], accS[:])
                ps_dt = ps_s.tile([128, 1024], f32, tag="ps_s", name="ps_s")
                nc.tensor.matmul(ps_dt[0:1, 0:512], ones_kv[:], gfold[:],
                                 start=True, stop=True)
                r_row = dpool.tile([1, 512], f32, tag="r_row", name="r_row")
                nc.vector.reciprocal_approx_fast(r_row[:], ps_dt[0:1, 0:512])
                rb = dpool.tile([128, 512], f32, tag="rb", name="rb")
                nc.gpsimd.partition_broadcast(rb[:], r_row[:])
                oT = opool.tile([128, 512], bf16, tag=f"oT_{h}", name=f"oT_{h}")
                nc.vector.tensor_mul(oT[:], oT_u[:], rb[:])
                oT_sb.setdefault(q4, {})[h] = oT

            yc_n = [0]

            def emit_y_chunk(q4, sl, j4):
                srow = slice((q4 * 4 + sl) * 128, (q4 * 4 + sl + 1) * 128)
                jcol = slice(j4 * 512, (j4 + 1) * 512)
                ps_y = ps_proj.tile([128, 512], f32, tag="ps_proj",
                                    name="ps_proj")
                for h in range(HPC):
                    nc.tensor.matmul(
                        ps_y[:], oT_sb[q4][h][:, sl * 128:(sl + 1) * 128],
                        wo_sb[h][:, jcol], start=(h == 0), stop=(h == HPC - 1))
                yc_n[0] += 1
                ych = ypool.tile([128, 512], bf16, tag=f"ych{yc_n[0] % 2}",
                                 name="ych")
                if yc_n[0] % 2 == 0:
                    nc.scalar.copy(ych[:], ps_y[:])
                else:
                    nc.vector.tensor_copy(ych[:], ps_y[:])
                nc.sync.dma_start(y[srow, jcol], ych[:])

            # ================= merged pipeline over columns =================
            y_queue = []
            for s4 in range(SQ):
                scol = slice(s4 * 512, (s4 + 1) * 512)
                xcol = xcols.pop(s4)

                def proj_mms(ps, w_sb, mm, ks):
                    for k in ks:
                        nc.tensor.matmul(
                            ps[:],
                            w_sb[k // 4][:, k % 4, mm * 128:(mm + 1) * 128],
                            xcol[k // 4][:, k % 4, :],
                            start=(k == 0), stop=(k == KC - 1))

                def rope_tail(ps, b_sb, mm, dtile):
                    q1 = rope.tile([128, 512], bf16, tag="q1", name="q1")
                    nc.scalar.activation(q1[:], ps[:], Act.Identity,
                                         bias=b_sb[:, mm:mm + 1])
                    # pair-swap halves via SBUF->SBUF DMA (partition
                    # shifts are not expressible on DVE/ACT lanes)
                    qsw = rope.tile([128, 512], bf16, tag="qsw", name="qsw")
                    nc.sync.dma_start(qsw[0:64], q1[64:128])
                    nc.sync.dma_start(qsw[64:128], q1[0:64])
                    tsw = rope.tile([128, 512], bf16, tag="tsw", name="tsw")
                    nc.vector.tensor_mul(tsw[:], qsw[:], sin_sb[:, scol])
                    # qsw is dead after tsw; reuse it for the cosine term
                    nc.vector.tensor_mul(qsw[:], q1[:], cos_sb[:, scol])
                    nc.vector.tensor_add(dtile[:], qsw[:], tsw[:])

                def q_dtile(mm, is_q):
                    if not is_q:
                        return kT_sb[mm][s4]
                    if inseg:
                        cur_qT[mm] = qp.tile([128, 512], bf16, tag=f"qTc_{mm}",
                                             name=f"qTc_{mm}")
                        return cur_qT[mm]
                    return qT_sb[mm][s4]

                # K then Q: out[dk, s] with RoPE (K first: scores read kT)
                for (w_sb, b_sb, is_q) in ((wk_sb, bk_sb, False),
                                           (wq_sb, bq_sb, True)):
                    if s4 == 0 and not is_q:
                        # startup: half-contraction interleave lets the PE
                        # begin with 2MB in SBUF instead of 4MB
                        ps_h = {}
                        for mm, half in ((0, 0), (1, 0), (0, 1), (2, 0),
                                         (1, 1), (3, 0), (2, 1), (3, 1)):
                            if half == 0:
                                ps_h[mm] = ps_proj.tile([128, 512], f32,
                                                        tag="ps_proj",
                                                        name="ps_proj")
                                proj_mms(ps_h[mm], w_sb, mm, range(KC // 2))
                            else:
                                proj_mms(ps_h[mm], w_sb, mm, range(KC // 2, KC))
                                rope_tail(ps_h.pop(mm), b_sb, mm,
                                          q_dtile(mm, is_q))
                        continue
                    for mm in range(HPC):
                        ps = ps_proj.tile([128, 512], f32, tag="ps_proj",
                                          name="ps_proj")
                        proj_mms(ps, w_sb, mm, range(KC))
                        rope_tail(ps, b_sb, mm, q_dtile(mm, is_q))

                # next column's x prefetch: issued after the K/Q swap DMAs so
                # the latency-critical RoPE swaps aren't queued behind 2MB
                if s4 + 1 < SQ:
                    xcols[s4 + 1] = [xpool.tile([128, 4, 512], bf16,
                                                tag=f"x_{i}", name=f"x_{i}")
                                     for i in range(KP)]
                    for i in range(KP):
                        nc.sync.dma_start(xcols[s4 + 1][i][:], xT[s4 + 1, i])

                # V: out[s, dk-shard], natural layout
                for sl in range(4):
                    s16 = s4 * 4 + sl
                    ps = ps_proj.tile([128, 512], f32, tag="ps_proj",
                                      name="ps_proj")
                    for k in range(KC):
                        nc.tensor.matmul(
                            ps[:],
                            xcol[k // 4][:, k % 4, sl * 128:(sl + 1) * 128],
                            wv_sb[k // 4][:, k % 4, :],
                            start=(k == 0), stop=(k == KC - 1))
                    nc.vector.tensor_add(v_sb[s16][:], ps[:], bvb_sb[:])

                # attention for columns whose k/v just became complete,
                # with previous columns' output rows interleaved between
                # heads to keep the PE fed during exp/mul latency.
                for q4 in seg_attns[s4]:
                    for h in range(HPC):
                        emit_attention_head(q4, h, y_queue)
                    for sl in range(4):
                        for j4 in range(JQ):
                            y_queue.append((q4, sl, j4))

                # prefetch m2 for the next segment's attention columns
                # (emitted last so the in-order queues never stall on it)
                if s4 + 1 < SQ:
                    for q4 in seg_attns[s4 + 1]:
                        load_m2(q4)

            # final flush: lag each chunk's last-head matmul one chunk behind
            # so the PE keeps running while the last attention head's
            # normalization chain drains
            pend_y = None

            def flush_finish(ps_y, q4, sl, j4):
                nc.tensor.matmul(
                    ps_y[:], oT_sb[q4][HPC - 1][:, sl * 128:(sl + 1) * 128],
                    wo_sb[HPC - 1][:, (j4) * 512:(j4 + 1) * 512],
                    start=False, stop=True)
                yc_n[0] += 1
                ych = ypool.tile([128, 512], bf16, tag=f"ych{yc_n[0] % 2}",
                                 name="ych")
                if yc_n[0] % 2 == 0:
                    nc.scalar.copy(ych[:], ps_y[:])
                else:
                    nc.vector.tensor_copy(ych[:], ps_y[:])
                nc.sync.dma_start(
                    y[(q4 * 4 + sl) * 128:(q4 * 4 + sl + 1) * 128,
                      j4 * 512:(j4 + 1) * 512], ych[:])

            while y_queue:
                q4, sl, j4 = y_queue.pop(0)
                ps_y = ps_proj.tile([128, 512], f32, tag="ps_proj",
                                    name="ps_proj")
                for h in range(HPC - 1):
                    nc.tensor.matmul(
                        ps_y[:], oT_sb[q4][h][:, sl * 128:(sl + 1) * 128],
                        wo_sb[h][:, j4 * 512:(j4 + 1) * 512],
                        start=(h == 0), stop=False)
                if pend_y is not None:
                    flush_finish(*pend_y)
                pend_y = (ps_y, q4, sl, j4)
            if pend_y is not None:
                flush_finish(*pend_y)

    nc.compile()
    return nc


def _rope_perm():
    """Within each head's 128 rows: evens first, then odds."""
    base = np.concatenate([np.arange(0, 128, 2), np.arange(1, 128, 2)])
    return np.concatenate([h * 128 + base for h in range(HPC)])


def _blk(a):
    """[R, C] -> [C//512, R//512, 128, 4, 512] packed contiguous blocks.

    Block [c4, i, :, j, :] = a[(4*i+j)*128:(4*i+j+1)*128, c4*512:(c4+1)*512].
    """
    r, c = a.shape
    return np.ascontiguousarray(
        a.reshape(r // 512, 4, 128, c // 512, 512).transpose(3, 0, 2, 1, 4))


def _wpack(a):
    """[R, C] -> [R//512, 128, 4, C]: pack 4 row-chunks per tile."""
    r, c = a.shape
    return np.ascontiguousarray(
        a.reshape(r // 512, 4, 128, c).transpose(0, 2, 1, 3))


def prepare_inputs(x, freqs, hard_mask, soft_mask, wq, bq, wk, bk, wv, bv, wo,
                   s_len=S):
    """Host-side shard + layout prep.  Returns one in_map per core."""
    perm = _rope_perm()
    cos = np.cos(np.asarray(freqs, np.float32))   # [S, 64]
    sin = np.sin(np.asarray(freqs, np.float32))
    cosp = np.ascontiguousarray(
        np.concatenate([cos.T, cos.T], axis=0)).astype(BF16)     # [128, S]
    sinp = np.ascontiguousarray(
        np.concatenate([-sin.T, sin.T], axis=0)).astype(BF16)
    hard = np.asarray(hard_mask, np.float32).reshape(s_len, s_len)
    soft = np.asarray(soft_mask, np.float32).reshape(s_len, s_len)
    m2t = _blk((hard * (soft + 1e-6)).T.astype(BF16))

    xT = [_blk(np.asarray(x[b], np.float32).T.astype(BF16)) for b in range(B)]

    per_group = []
    for hg in range(4):
        rows = slice(DSH * hg, DSH * (hg + 1))
        wq_sh = np.asarray(wq, np.float32)[rows][perm]
        wk_sh = np.asarray(wk, np.float32)[rows][perm]
        wv_sh = np.asarray(wv, np.float32)[rows]
        per_group.append({
            "wqT": _wpack(np.ascontiguousarray(wq_sh.T).astype(BF16)),
            "wkT": _wpack(np.ascontiguousarray(wk_sh.T).astype(BF16)),
            "wvT": _wpack(np.ascontiguousarray(wv_sh.T).astype(BF16)),
            "woT": np.ascontiguousarray(
                np.asarray(wo, np.float32)[:, rows].T).astype(BF16),
            "bqp": np.ascontiguousarray(
                np.asarray(bq, np.float32)[rows][perm].reshape(HPC, 128).T),
            "bkp": np.ascontiguousarray(
                np.asarray(bk, np.float32)[rows][perm].reshape(HPC, 128).T),
            "bvb": np.ascontiguousarray(np.broadcast_to(
                np.asarray(bv, np.float32)[rows][None, :], (128, DSH))),
        })

    in_maps = []
    for core in range(N_CORES):
        b, hg = core // 4, core % 4
        m = {"xT": xT[b], "cosp": cosp, "sinp": sinp, "m2t": m2t}
        m.update(per_group[hg])
        in_maps.append(m)
    return in_maps


def kernel(x, freqs, hard_mask, soft_mask, wq, bq, wk, bk, wv, bv, wo, bo,
           _trace=False, _tmpdir=None):
    s_len = x.shape[1]
    in_maps = prepare_inputs(x, freqs, hard_mask, soft_mask, wq, bq, wk, bk,
                             wv, bv, wo, s_len=s_len)
    m2b = in_maps[0]["m2t"]  # [SQ, NKV//4, 128, 4, 512]
    keep = []
    trunc = []
    for q4 in range(m2b.shape[0]):
        krow, trow = [], []
        for kv in range(m2b.shape[1] * 4):
            blk = m2b[q4, kv // 4, :, kv % 4]
            nz = np.flatnonzero(np.any(blk != 0, axis=0))
            krow.append(nz.size > 0)
            # exact zero-prefix width, 128-col granularity
            trow.append(int(nz[0]) // 128 * 128 if nz.size else 0)
        if not any(k and t == 0 for k, t in zip(krow, trow)):
            trow = [0] * len(trow)   # need one full-width block per column
        keep.append(tuple(krow))
        trunc.append(tuple(trow))
    keep, trunc = tuple(keep), tuple(trunc)
    ckey = (s_len, keep, trunc)
    if ckey not in _NC_CACHE:
        _NC_CACHE[ckey] = build_bass(s_len, keep, trunc)
    nc = _NC_CACHE[ckey]
    kwargs = {}
    if _trace:
        kwargs = {"trace": True, "tmpdir": _tmpdir}
    res = run_bass_kernel_spmd(nc, in_maps, core_ids=list(range(N_CORES)),
                               **kwargs)
    bo32 = np.asarray(bo, np.float32)
    out = np.empty((B, s_len, D), np.float32)
    for b in range(B):
        acc = res.results[4 * b]["y"].astype(np.float32)
        for hg in range(1, 4):
            acc = acc + res.results[4 * b + hg]["y"].astype(np.float32)
        out[b] = acc + bo32[None, :]
    kernel.last_result = res
    return out


# revision 45
# speedup vs baseline: 1.1689x; 1.0051x over previous
"""Trainium2 Bass kernel for nn_Attention_39049842655427.

Multi-head attention (RoPE + hard mask + soft gate mask) over 8
NeuronCores: data-parallel over batch (2) x tensor-parallel over heads
(16 heads -> 4 per core).  Each core computes q/k/v projections for its
4 heads, the head-sharded attention, and a partial output projection
(wo row-sharded); the host sums the 4 partials per batch and adds bo.

Math notes (exact up to float rounding):
  reference:  e = exp(s)*hard ; a1 = e/sum(e) ; a2 = a1*soft
              attn = a2/(sum(a2)+1e-6) ; out = attn @ v
  identity:   attn = f / (F + 1e-6*E),  f = e*hard*soft,
              F = sum(f), E = sum(e*hard)
  kernel:     g = exp(s) * M2,  M2 = hard*(soft+1e-6)
              => sum(g) = F + 1e-6*E exactly; numerator uses g instead
              of f, an O(1e-6) perturbation of attn.

Pipeline structure (single merged stream, no phase split): for each
512-wide token column s4: project K/Q/V for that column, then run the
attention for query column q4=s4 (causal => only needs k/v up to s4),
then emit the output-projection rows of column s4-1 interleaved between
attention heads.  This spreads ACT(exp)/DVE load evenly under the PE's
GEMM stream.  Scores are computed transposed (s[kv,q]) so attn@v needs
no transpose; exps are batched over two PSUM banks ([128,1024]); the
softmax denominator is one ones-matmul per head after a DVE tree
reduce; the AV accumulator is evacuated unnormalized so the PSUM bank
recycles without waiting on the reciprocal chain.  RoPE pairs are
pre-permuted so the rotation partner is a partition offset of 64
(SBUF->SBUF DMA).  Zero blocks of M2 are skipped and zero PREFIXES of
diagonal blocks are truncated to 384/256/128-wide matmuls (both exact
and data-adaptive: read from the actual mask and baked into the
compiled program; dense masks fall back to the full schedule).
"""

import math
import sys

import numpy as np
import ml_dtypes

if "/opt/trn_rl_repo" not in sys.path:
    sys.path.append("/opt/trn_rl_repo")

import concourse.bass as bass  # noqa: E402,F401
import concourse.tile as tile  # noqa: E402
from concourse import bacc, bass_isa, mybir  # noqa: E402
from concourse.bass_utils import run_bass_kernel_spmd  # noqa: E402

B, S, D, H, DK = 2, 2048, 2048, 16, 128
N_CORES = 8
HPC = 4          # heads per core
DSH = HPC * DK   # 512, d-shard per core

BF16 = ml_dtypes.bfloat16

_NC_CACHE = {}


def build_bass(s_len=S, keep=None, trunc=None):
    """Build the SPMD single-core program (same NEFF on all 8 cores)."""
    f32 = mybir.dt.float32
    bf16 = mybir.dt.bfloat16
    KC = D // 128          # contraction chunks for projections
    SQ = s_len // 512      # 512-wide q/s chunks
    NKV = s_len // 128     # 128-row kv chunks
    JQ = D // 512          # output-column chunks
    KP = KC // 4           # packed weight/x tiles per column
    NQ = NKV // 4          # kv quads
    if keep is None:
        keep = tuple(tuple(True for _ in range(NKV)) for _ in range(SQ))
    if trunc is None:
        trunc = tuple(tuple(0 for _ in range(NKV)) for _ in range(SQ))
    kept_l = {q4: [kv for kv in range(NKV) if keep[q4][kv]] for q4 in range(SQ)}
    for q4 in range(SQ):
        assert kept_l[q4], "fully masked query column not supported"
    # attn(q4) can run once all its k/v chunks are projected
    sched = {q4: max(q4, max(kept_l[q4]) // 4) for q4 in range(SQ)}
    seg_attns = {s4: [q4 for q4 in range(SQ) if sched[q4] == s4]
                 for s4 in range(SQ)}
    # causal fast path: every column's attention runs in its own segment,
    # so qT tiles can live in a small rotating per-segment pool
    inseg = all(sched[q4] == q4 for q4 in range(SQ))

    nc = bacc.Bacc("TRN2", target_bir_lowering=False, debug=False,
                   num_devices=N_CORES)

    xT = nc.dram_tensor("xT", [SQ, KP, 128, 4, 512], bf16, kind="ExternalInput").ap()
    wqT = nc.dram_tensor("wqT", [KP, 128, 4, DSH], bf16, kind="ExternalInput").ap()
    wkT = nc.dram_tensor("wkT", [KP, 128, 4, DSH], bf16, kind="ExternalInput").ap()
    wvT = nc.dram_tensor("wvT", [KP, 128, 4, DSH], bf16, kind="ExternalInput").ap()
    woT = nc.dram_tensor("woT", [DSH, D], bf16, kind="ExternalInput").ap()
    bqp = nc.dram_tensor("bqp", [128, HPC], f32, kind="ExternalInput").ap()
    bkp = nc.dram_tensor("bkp", [128, HPC], f32, kind="ExternalInput").ap()
    bvb = nc.dram_tensor("bvb", [128, DSH], f32, kind="ExternalInput").ap()
    cosp = nc.dram_tensor("cosp", [128, s_len], bf16, kind="ExternalInput").ap()
    sinp = nc.dram_tensor("sinp", [128, s_len], bf16, kind="ExternalInput").ap()
    m2t = nc.dram_tensor("m2t", [SQ, NQ, 128, 4, 512], bf16, kind="ExternalInput").ap()
    y = nc.dram_tensor("y", [s_len, D], bf16, kind="ExternalOutput").ap()

    Act = mybir.ActivationFunctionType
    inv_sqrt_dk = 1.0 / math.sqrt(DK)

    with tile.TileContext(nc) as tc:
        with (
            tc.tile_pool(name="consts", bufs=1) as consts,
            tc.tile_pool(name="kvp", bufs=1) as kvp,
            tc.tile_pool(name="wpool", bufs=1) as wpool,
            tc.tile_pool(name="xpool", bufs=2) as xpool,
            tc.tile_pool(name="m2pool", bufs=1) as m2pool,
            tc.tile_pool(name="opool", bufs=2) as opool,
            tc.tile_pool(name="rope", bufs=2) as rope,
            tc.tile_pool(name="attw", bufs=2) as attw,
            tc.tile_pool(name="accp", bufs=2) as accp,
            tc.tile_pool(name="dpool", bufs=2) as dpool,
            tc.tile_pool(name="ypool", bufs=2) as ypool,
            tc.tile_pool(name="qp", bufs=2) as qp,
            tc.tile_pool(name="ps_proj", bufs=2, space="PSUM") as ps_proj,
            tc.tile_pool(name="ps_s", bufs=2, space="PSUM") as ps_s,
            tc.tile_pool(name="ps_o", bufs=2, space="PSUM") as ps_o,
        ):
            # ---- persistent tiles ----
            warm = consts.tile([128, 512], bf16, tag="warm", name="warm")
            nc.vector.memset(warm, 0.0)
            ones_kv = consts.tile([128, 1], bf16, tag="ones_kv", name="ones_kv")
            nc.vector.memset(ones_kv, 1.0)
            kT_sb = [[kvp.tile([128, 512], bf16, tag=f"kT_{h}_{c}",
                               name=f"kT_{h}_{c}") for c in range(SQ)]
                     for h in range(HPC)]
            if inseg:
                qT_sb = None        # rotating per-segment tiles (see loop)
            else:
                qT_sb = [[kvp.tile([128, 512], bf16, tag=f"qT_{h}_{c}",
                                   name=f"qT_{h}_{c}") for c in range(SQ)]
                         for h in range(HPC)]
            cur_qT = {}             # h -> current segment's qT tile
            v_sb = [kvp.tile([128, DSH], bf16, tag=f"v_{i}", name=f"v_{i}")
                    for i in range(NKV)]
            wo_sb = [consts.tile([128, D], bf16, tag=f"wo_{h}", name=f"wo_{h}")
                     for h in range(HPC)]
            wq_sb = [wpool.tile([128, 4, DSH], bf16, tag=f"wq_{i}", name=f"wq_{i}")
                     for i in range(KP)]
            wk_sb = [wpool.tile([128, 4, DSH], bf16, tag=f"wk_{i}", name=f"wk_{i}")
                     for i in range(KP)]
            wv_sb = [wpool.tile([128, 4, DSH], bf16, tag=f"wv_{i}", name=f"wv_{i}")
                     for i in range(KP)]

            # ---- startup DMAs: 3 queues drained round-robin by the DMA
            # engines, so each queue is ordered by first-need and the
            # phases stay roughly aligned across queues ----
            xcols = {0: [xpool.tile([128, 4, 512], bf16, tag=f"x_{i}",
                                    name=f"x_{i}") for i in range(KP)]}
            cos_sb = consts.tile([128, s_len], bf16, tag="cos", name="cos")
            sin_sb = consts.tile([128, s_len], bf16, tag="sin", name="sin")
            bq_sb = consts.tile([128, HPC], f32, tag="bq", name="bq")
            bk_sb = consts.tile([128, HPC], f32, tag="bk", name="bk")
            bvb_sb = consts.tile([128, DSH], f32, tag="bvb", name="bvb")
            # phase 1: K inputs on the two HWDGE queues, consts on gpsimd
            for i in range(KP):
                nc.sync.dma_start(xcols[0][i][:], xT[0, i])
                nc.scalar.dma_start(wk_sb[i][:], wkT[i])
            nc.gpsimd.dma_start(bq_sb[:], bqp[:])
            nc.gpsimd.dma_start(bk_sb[:], bkp[:])
            nc.gpsimd.dma_start(cos_sb[:], cosp[:])
            nc.gpsimd.dma_start(sin_sb[:], sinp[:])
            # phase 2: Q weights split across the HWDGE queues; the rest on
            # gpsimd ordered by first need (keeps the sync queue short so the
            # latency-critical RoPE swap DMAs aren't stuck behind bulk loads)
            nc.sync.dma_start(wq_sb[0][:], wqT[0])
            nc.sync.dma_start(wq_sb[2][:], wqT[2])
            nc.scalar.dma_start(wq_sb[1][:], wqT[1])
            nc.scalar.dma_start(wq_sb[3][:], wqT[3])
            nc.gpsimd.dma_start(bvb_sb[:], bvb[:])
            for i in range(KP):
                nc.gpsimd.dma_start(wv_sb[i][:], wvT[i])

            m2_tiles = {}  # q4 -> {quad: tile}

            def load_m2(q4, eng=None):
                eng = eng or nc.gpsimd
                m2_tiles[q4] = {}
                for i in range(NQ):
                    if any(keep[q4][4 * i + j] for j in range(4)):
                        t = m2pool.tile([128, 4, 512], bf16, tag=f"m2_{i}",
                                        name=f"m2_{i}")
                        eng.dma_start(t[:], m2t[q4, i])
                        m2_tiles[q4][i] = t

            # phase 4: first attention masks + output weights
            for q4 in seg_attns[0]:
                load_m2(q4, eng=nc.gpsimd)
            for h in range(HPC):
                nc.gpsimd.dma_start(wo_sb[h][:], woT[h * 128:(h + 1) * 128, :])

            # ---- HAM warm-up: open the PE clock gate while DMAs land ----
            ps_warm = ps_proj.tile([128, 512], f32, tag="ps_proj", name="ps_proj")
            for i in range(12):
                nc.tensor.matmul(ps_warm[:], warm[:, 0:128], warm[:],
                                 start=(i == 0), stop=(i == 11))

            oT_sb = {}

            def emit_attention_head(q4, h, y_queue=()):
                m2c = m2_tiles[q4]
                qT = cur_qT[h] if inseg else qT_sb[h][q4]
                # pieces: (kv, zero-prefix offset). Build groups:
                #   fp: tile-adjacent pairs of full-width pieces
                #   fs: leftover full singles (first one seeds the accS chain)
                #   tg: truncated pieces packed into <=1024-wide psum groups
                pieces = [(kv, trunc[q4][kv]) for kv in kept_l[q4]]
                fulls = [kv for kv, t in pieces if t == 0]
                truncs = [(kv, t) for kv, t in pieces if t > 0]
                fp, fs = [], []
                i = 0
                while i < len(fulls):
                    if (i + 1 < len(fulls) and fulls[i + 1] == fulls[i] + 1
                            and fulls[i] // 4 == fulls[i + 1] // 4):
                        fp.append([(fulls[i], 0), (fulls[i + 1], 0)])
                        i += 2
                    else:
                        fs.append([(fulls[i], 0)])
                        i += 1
                # pack truncated pieces into the two 512-wide PSUM banks of a
                # score tile: a matmul output must not cross a bank boundary
                tg = []   # each: ([items], [remA, remB])
                for kv, t in sorted(truncs, key=lambda p: p[1]):
                    w = 512 - t
                    placed = False
                    if tg:
                        items, rem = tg[-1]
                        for bnk in (0, 1):
                            if rem[bnk] >= w:
                                items.append((kv, t, 512 * bnk + 512 - rem[bnk], w))
                                rem[bnk] -= w
                                placed = True
                                break
                    if not placed:
                        tg.append(([(kv, t, 0, w)], [512 - w, 512]))
                assert fulls, "query column with no full-width kv block"
                fp = [[(a, 0, 0, 512), (b, 0, 512, 512)] for (a, _), (b, _) in fp]
                fs = [[(g[0][0], 0, 0, 512)] for g in fs]
                tg = [items for items, _ in tg]
                seq = fp[:1] + fs + tg + fp[1:]
                n_av = len(pieces)
                ng = len(seq)
                # positions (after which score-group) to slot a y chunk so the
                # PE always has ready work while exp/mul latency drains
                ypos = {max(1, math.ceil(ng * k / 4)) for k in (1, 2, 3, 4)}
                ps_oT = ps_o.tile([128, 512], f32, tag="ps_o", name="ps_o")
                accA = accS = None          # pair chain / single+trunc chain
                av_i = 0
                pend = []      # accumulator adds deferred to lag behind AV
                pend_av = []   # AV emission lagged one group behind scores
                pair_i = 0

                def emit_av():
                    nonlocal av_i
                    g2, lay = pend_av.pop(0)
                    for kv, off, c, w in lay:
                        nc.tensor.matmul(
                            ps_oT[:, off:512],
                            v_sb[kv][:, h * 128:(h + 1) * 128],
                            g2[:, c:c + w],
                            start=(av_i == 0), stop=(av_i == n_av - 1))
                        av_i += 1

                for gi, grp in enumerate(seq):
                    lay = grp  # (kv, off, psum col, width) per piece
                    tot = max(c + w for _, _, c, w in lay)
                    is_fpair = (len(grp) == 2 and grp[0][1] == 0
                                and grp[1][1] == 0)
                    psc = ps_s.tile([128, 1024], f32, tag="ps_s", name="ps_s")
                    for kv, off, c, w in lay:
                        nc.tensor.matmul(
                            psc[:, c:c + w],
                            kT_sb[h][kv // 4][:, (kv % 4) * 128:(kv % 4 + 1) * 128],
                            qT[:, off:512], start=True, stop=True)
                    if is_fpair:
                        if accA is None:
                            g = accA = accp.tile([128, 1024], bf16, tag="gaccA",
                                                 name="gaccA")
                            chain = None
                        else:
                            g = attw.tile([128, 1024], bf16,
                                          tag=f"g{pair_i % 2}", name="g")
                            chain = 'A'
                        pair_i += 1
                    elif len(grp) == 1 and grp[0][1] == 0:
                        if accS is None:
                            g = accS = accp.tile([128, 512], bf16, tag="gaccS",
                                                 name="gaccS")
                            chain = None
                        else:
                            g = attw.tile([128, 512], bf16, tag="gs", name="gs")
                            chain = 'S'
                    else:
                        g = attw.tile([128, 1024], bf16,
                                      tag=f"g{pair_i % 2}", name="g")
                        pair_i += 1
                        chain = 'T'
                    nc.scalar.activation(g[:, 0:tot], psc[:, 0:tot], Act.Exp,
                                         scale=inv_sqrt_dk)
                    if is_fpair and grp[1][0] == grp[0][0] + 1:
                        kv0 = grp[0][0]
                        nc.vector.tensor_mul(
                            g[:, 0:1024], g[:, 0:1024],
                            m2c[kv0 // 4][:, kv0 % 4:kv0 % 4 + 2, :])
                    else:
                        for kv, off, c, w in lay:
                            nc.vector.tensor_mul(
                                g[:, c:c + w], g[:, c:c + w],
                                m2c[kv // 4][:, kv % 4, off:512])
                    pend_av.append((g, lay))
                    if gi + 1 in ypos and y_queue:
                        emit_y_chunk(*y_queue.pop(0))
                    if len(pend_av) > 1:
                        emit_av()
                    if chain is not None:
                        pend.append((g, lay, chain))
                    # drain pending accumulator adds (lag keeps AV unblocked)
                    while len(pend) > 1 or (gi == ng - 1 and pend):
                        g2, lay2, ch = pend.pop(0)
                        if ch == 'A':
                            nc.vector.tensor_add(accA[:], accA[:], g2[:])
                        elif ch == 'S':
                            nc.vector.tensor_add(accS[:], accS[:], g2[:])
                        else:
                            for kv, off, c, w in lay2:
                                nc.vector.tensor_add(
                                    accS[:, off:512], accS[:, off:512],
                                    g2[:, c:c + w])
                while pend_av:
                    emit_av()
                # evacuate the AV accumulator immediately (frees the PSUM
                # bank for the next head without waiting on the denominator
                # chain); normalization happens later from SBUF
                oT_u = attw.tile([128, 512], bf16, tag="otu", name="otu")
                if h % 2 == 0:
                    nc.scalar.copy(oT_u[:], ps_oT[:])
                else:
                    nc.vector.tensor_copy(oT_u[:], ps_oT[:])
                # fold chains into one [128,512] bf16 row-block sum, then a
                # single ones-matmul computes the denominator row (1 PE
                # instruction per head; psum slot borrowed from ps_s pool)
                gfold = dpool.tile([128, 512], bf16, tag="gfold", name="gfold")
                if accA is not None:
                    nc.vector.tensor_add(gfold[:], accA[:, 0:512],
                                         accA[:, 512:1024])
                    if accS is not None:
                        nc.vector.tensor_add(gfold[:], gfold[:], accS[:])
                else:
                    nc.vector.tensor_copy(gfold[:], accS[:])
                ps_dt = ps_s.tile([128, 1024], f32, tag="ps_s", name="ps_s")
                nc.tensor.matmul(ps_dt[0:1, 0:512], ones_kv[:], gfold[:],
                                 start=True, stop=True)
                r_row = dpool.tile([1, 512], f32, tag="r_row", name="r_row")
                nc.vector.reciprocal_approx_fast(r_row[:], ps_dt[0:1, 0:512])
                rb = dpool.tile([128, 512], f32, tag="rb", name="rb")
                nc.gpsimd.partition_broadcast(rb[:], r_row[:])
                oT = opool.tile([128, 512], bf16, tag=f"oT_{h}", name=f"oT_{h}")
                nc.vector.tensor_mul(oT[:], oT_u[:], rb[:])
                oT_sb.setdefault(q4, {})[h] = oT

            yc_n = [0]

            def emit_y_chunk(q4, sl, j4):
                srow = slice((q4 * 4 + sl) * 128, (q4 * 4 + sl + 1) * 128)
                jcol = slice(j4 * 512, (j4 + 1) * 512)
                ps_y = ps_proj.tile([128, 512], f32, tag="ps_proj",
                                    name="ps_proj")
                for h in range(HPC):
                    nc.tensor.matmul(
                        ps_y[:], oT_sb[q4][h][:, sl * 128:(sl + 1) * 128],
                        wo_sb[h][:, jcol], start=(h == 0), stop=(h == HPC - 1))
                yc_n[0] += 1
                ych = ypool.tile([128, 512], bf16, tag=f"ych{yc_n[0] % 2}",
                                 name="ych")
                if yc_n[0] % 2 == 0:
                    nc.scalar.copy(ych[:], ps_y[:])
                else:
                    nc.vector.tensor_copy(ych[:], ps_y[:])
                nc.sync.dma_start(y[srow, jcol], ych[:])

            # ================= merged pipeline over columns =================
            y_queue = []
            for s4 in range(SQ):
                scol = slice(s4 * 512, (s4 + 1) * 512)
                xcol = xcols.pop(s4)

                def proj_mms(ps, w_sb, mm, ks):
                    for k in ks:
                        nc.tensor.matmul(
                            ps[:],
                            w_sb[k // 4][:, k % 4, mm * 128:(mm + 1) * 128],
                            xcol[k // 4][:, k % 4, :],
                            start=(k == 0), stop=(k == KC - 1))

                def rope_tail(ps, b_sb, mm, dtile):
                    q1 = rope.tile([128, 512], bf16, tag="q1", name="q1")
                    nc.scalar.activation(q1[:], ps[:], Act.Identity,
                                         bias=b_sb[:, mm:mm + 1])
                    # pair-swap halves via SBUF->SBUF DMA (partition
                    # shifts are not expressible on DVE/ACT lanes)
                    qsw = rope.tile([128, 512], bf16, tag="qsw", name="qsw")
                    nc.sync.dma_start(qsw[0:64], q1[64:128])
                    nc.sync.dma_start(qsw[64:128], q1[0:64])
                    tsw = rope.tile([128, 512], bf16, tag="tsw", name="tsw")
                    nc.vector.tensor_mul(tsw[:], qsw[:], sin_sb[:, scol])
                    # qsw is dead after tsw; reuse it for the cosine term
                    nc.vector.tensor_mul(qsw[:], q1[:], cos_sb[:, scol])
                    nc.vector.tensor_add(dtile[:], qsw[:], tsw[:])

                def q_dtile(mm, is_q):
                    if not is_q:
                        return kT_sb[mm][s4]
                    if inseg:
                        cur_qT[mm] = qp.tile([128, 512], bf16, tag=f"qTc_{mm}",
                                             name=f"qTc_{mm}")
                        return cur_qT[mm]
                    return qT_sb[mm][s4]

                # K then Q: out[dk, s] with RoPE (K first: scores read kT)
                for (w_sb, b_sb, is_q) in ((wk_sb, bk_sb, False),
                                           (wq_sb, bq_sb, True)):
                    if s4 == 0 and not is_q:
                        # startup: half-contraction interleave lets the PE
                        # begin with 2MB in SBUF instead of 4MB
                        ps_h = {}
                        for mm, half in ((0, 0), (1, 0), (0, 1), (2, 0),
                                         (1, 1), (3, 0), (2, 1), (3, 1)):
                            if half == 0:
                                ps_h[mm] = ps_proj.tile([128, 512], f32,
                                                        tag="ps_proj",
                                                        name="ps_proj")
                                proj_mms(ps_h[mm], w_sb, mm, range(KC // 2))
                            else:
                                proj_mms(ps_h[mm], w_sb, mm, range(KC // 2, KC))
                                rope_tail(ps_h.pop(mm), b_sb, mm,
                                          q_dtile(mm, is_q))
                        continue
                    for mm in range(HPC):
                        ps = ps_proj.tile([128, 512], f32, tag="ps_proj",
                                          name="ps_proj")
                        proj_mms(ps, w_sb, mm, range(KC))
                        rope_tail(ps, b_sb, mm, q_dtile(mm, is_q))

                # next column's x prefetch: issued after the K/Q swap DMAs so
                # the latency-critical RoPE swaps aren't queued behind 2MB
                if s4 + 1 < SQ:
                    xcols[s4 + 1] = [xpool.tile([128, 4, 512], bf16,
                                                tag=f"x_{i}", name=f"x_{i}")
                                     for i in range(KP)]
                    for i in range(KP):
                        nc.sync.dma_start(xcols[s4 + 1][i][:], xT[s4 + 1, i])

                # V: out[s, dk-shard], natural layout
                for sl in range(4):
                    s16 = s4 * 4 + sl
                    ps = ps_proj.tile([128, 512], f32, tag="ps_proj",
                                      name="ps_proj")
                    for k in range(KC):
                        nc.tensor.matmul(
                            ps[:],
                            xcol[k // 4][:, k % 4, sl * 128:(sl + 1) * 128],
                            wv_sb[k // 4][:, k % 4, :],
                            start=(k == 0), stop=(k == KC - 1))
                    nc.vector.tensor_add(v_sb[s16][:], ps[:], bvb_sb[:])

                # attention for columns whose k/v just became complete,
                # with previous columns' output rows interleaved between
                # heads to keep the PE fed during exp/mul latency.
                for q4 in seg_attns[s4]:
                    for h in range(HPC):
                        emit_attention_head(q4, h, y_queue)
                    for sl in range(4):
                        for j4 in range(JQ):
                            y_queue.append((q4, sl, j4))

                # prefetch m2 for the next segment's attention columns
                # (emitted last so the in-order queues never stall on it)
                if s4 + 1 < SQ:
                    for q4 in seg_attns[s4 + 1]:
                        load_m2(q4)

            # final flush: lag each chunk's last-head matmul one chunk behind
            # so the PE keeps running while the last attention head's
            # normalization chain drains
            pend_y = None

            def flush_finish(ps_y, q4, sl, j4):
                nc.tensor.matmul(
                    ps_y[:], oT_sb[q4][HPC - 1][:, sl * 128:(sl + 1) * 128],
                    wo_sb[HPC - 1][:, (j4) * 512:(j4 + 1) * 512],
                    start=False, stop=True)
                yc_n[0] += 1
                ych = ypool.tile([128, 512], bf16, tag=f"ych{yc_n[0] % 2}",
                                 name="ych")
                if yc_n[0] % 2 == 0:
                    nc.scalar.copy(ych[:], ps_y[:])
                else:
                    nc.vector.tensor_copy(ych[:], ps_y[:])
                nc.sync.dma_start(
                    y[(q4 * 4 + sl) * 128:(q4 * 4 + sl + 1) * 128,
                      j4 * 512:(j4 + 1) * 512], ych[:])

            while y_queue:
                q4, sl, j4 = y_queue.pop(0)
                ps_y = ps_proj.tile([128, 512], f32, tag="ps_proj",
                                    name="ps_proj")
                for h in range(HPC - 1):
                    nc.tensor.matmul(
                        ps_y[:], oT_sb[q4][h][:, sl * 128:(sl + 1) * 128],
                        wo_sb[h][:, j4 * 512:(j4 + 1) * 512],
                        start=(h == 0), stop=False)
                if pend_y is not None:
                    flush_finish(*pend_y)
                pend_y = (ps_y, q4, sl, j4)
            if pend_y is not None:
                flush_finish(*pend_y)

    nc.compile()
    return nc


def _rope_perm():
    """Within each head's 128 rows: evens first, then odds."""
    base = np.concatenate([np.arange(0, 128, 2), np.arange(1, 128, 2)])
    return np.concatenate([h * 128 + base for h in range(HPC)])


def _blk(a):
    """[R, C] -> [C//512, R//512, 128, 4, 512] packed contiguous blocks.

    Block [c4, i, :, j, :] = a[(4*i+j)*128:(4*i+j+1)*128, c4*512:(c4+1)*512].
    """
    r, c = a.shape
    return np.ascontiguousarray(
        a.reshape(r // 512, 4, 128, c // 512, 512).transpose(3, 0, 2, 1, 4))


def _wpack(a):
    """[R, C] -> [R//512, 128, 4, C]: pack 4 row-chunks per tile."""
    r, c = a.shape
    return np.ascontiguousarray(
        a.reshape(r // 512, 4, 128, c).transpose(0, 2, 1, 3))


def prepare_inputs(x, freqs, hard_mask, soft_mask, wq, bq, wk, bk, wv, bv, wo,
                   s_len=S):
    """Host-side shard + layout prep.  Returns one in_map per core."""
    perm = _rope_perm()
    cos = np.cos(np.asarray(freqs, np.float32))   # [S, 64]
    sin = np.sin(np.asarray(freqs, np.float32))
    cosp = np.ascontiguousarray(
        np.concatenate([cos.T, cos.T], axis=0)).astype(BF16)     # [128, S]
    sinp = np.ascontiguousarray(
        np.concatenate([-sin.T, sin.T], axis=0)).astype(BF16)
    hard = np.asarray(hard_mask, np.float32).reshape(s_len, s_len)
    soft = np.asarray(soft_mask, np.float32).reshape(s_len, s_len)
    m2t = _blk((hard * (soft + 1e-6)).T.astype(BF16))

    xT = [_blk(np.asarray(x[b], np.float32).T.astype(BF16)) for b in range(B)]

    per_group = []
    for hg in range(4):
        rows = slice(DSH * hg, DSH * (hg + 1))
        wq_sh = np.asarray(wq, np.float32)[rows][perm]
        wk_sh = np.asarray(wk, np.float32)[rows][perm]
        wv_sh = np.asarray(wv, np.float32)[rows]
        per_group.append({
            "wqT": _wpack(np.ascontiguousarray(wq_sh.T).astype(BF16)),
            "wkT": _wpack(np.ascontiguousarray(wk_sh.T).astype(BF16)),
            "wvT": _wpack(np.ascontiguousarray(wv_sh.T).astype(BF16)),
            "woT": np.ascontiguousarray(
                np.asarray(wo, np.float32)[:, rows].T).astype(BF16),
            "bqp": np.ascontiguousarray(
                np.asarray(bq, np.float32)[rows][perm].reshape(HPC, 128).T),
            "bkp": np.ascontiguousarray(
                np.asarray(bk, np.float32)[rows][perm].reshape(HPC, 128).T),
            "bvb": np.ascontiguousarray(np.broadcast_to(
                np.asarray(bv, np.float32)[rows][None, :], (128, DSH))),
        })

    in_maps = []
    for core in range(N_CORES):
        b, hg = core // 4, core % 4
        m = {"xT": xT[b], "cosp": cosp, "sinp": sinp, "m2t": m2t}
        m.update(per_group[hg])
        in_maps.append(m)
    return in_maps


def kernel(x, freqs, hard_mask, soft_mask, wq, bq, wk, bk, wv, bv, wo, bo,
           _trace=False, _tmpdir=None):
    s_len = x.shape[1]
    in_maps = prepare_inputs(x, freqs, hard_mask, soft_mask, wq, bq, wk, bk,
                             wv, bv, wo, s_len=s_len)
    m2b = in_maps[0]["m2t"]  # [SQ, NKV//4, 128, 4, 512]
    keep = []
    trunc = []
    for q4 in range(m2b.shape[0]):
        krow, trow = [], []
        for kv in range(m2b.shape[1] * 4):
            blk = m2b[q4, kv // 4, :, kv % 4]
            nz = np.flatnonzero(np.any(blk != 0, axis=0))
            krow.append(nz.size > 0)
            # exact zero-prefix width, 128-col granularity
            trow.append(int(nz[0]) // 128 * 128 if nz.size else 0)
        if not any(k and t == 0 for k, t in zip(krow, trow)):
            trow = [0] * len(trow)   # need one full-width block per column
        keep.append(tuple(krow))
        trunc.append(tuple(trow))
    keep, trunc = tuple(keep), tuple(trunc)
    ckey = (s_len, keep, trunc)
    if ckey not in _NC_CACHE:
        _NC_CACHE[ckey] = build_bass(s_len, keep, trunc)
    nc = _NC_CACHE[ckey]
    kwargs = {}
    if _trace:
        kwargs = {"trace": True, "tmpdir": _tmpdir}
    res = run_bass_kernel_spmd(nc, in_maps, core_ids=list(range(N_CORES)),
                               **kwargs)
    bo32 = np.asarray(bo, np.float32)
    out = np.empty((B, s_len, D), np.float32)
    for b in range(B):
        acc = res.results[4 * b]["y"].astype(np.float32)
        for hg in range(1, 4):
            acc = acc + res.results[4 * b + hg]["y"].astype(np.float32)
        out[b] = acc + bo32[None, :]
    kernel.last_result = res
    return out
